# revision 31
# baseline (speedup 1.0000x reference)
"""Mixtral attention (B=2, S=1024, H=4096, NH=32, NKV=8, D=128), GQA + RoPE +
causal mask + o_proj, tensor-parallel over heads across 8 TRN2 NeuronCores.

Sharding: core c owns Q heads 4c..4c+3 and KV head c. Each core computes its
heads' attention output and a partial o_proj product (rows of wo owned by its
heads); the host sums the 8 partials.

Layout strategy (per core):
  - host pre-transposes hidden -> hT [H, B*S] so the contraction dim (H) lies
    on SBUF partitions for QKV projections; weights pre-tiled on host into
    per-k-tile combined blocks wall[kt] = [128, 768] (4 q heads | K | V) so
    each arriving hT tile immediately feeds 6 matmuls.
  - QKV projection in two passes over k-halves (only half the hidden is
    SBUF-resident). Pass 1 is kt-outer/m-inner with 6 PSUM accumulators:
    the first n-sweep runs in lockstep with the hT DMA stream (hT loaded in
    column-halves so the first sweep needs only 320KB per k-tile).
  - RoPE is linear, so each pass ropes its partial directly into qk_roped
    (pass 2 adds); no full partial buffers needed for q/K. V keeps a bf16
    partial (part_v) and is transposed to natural [s, d] layout on PE.
  - scores computed transposed: S^T[k, q] = K-block^T.T @ Q^T. Softmax-over-k
    becomes: exp on ACT (scale folded in; padding mask folded into the
    per-partition bias operand), denominator via ones-vector matmul on PE.
    The four denominators of a (b=0,b=1) head pair are packed into rows
    0/32/64/96 of one SBUF tile so a SINGLE ACT reciprocal serves the pair
    (2 ACT table loads per slot instead of per-jc thrash); the reciprocal is
    broadcast to [128, q] by a selector matmul and multiplied into the
    unnormalized PV result in place.
  - causal mask: blocks strictly above the diagonal are skipped outright;
    diagonal 128x128 blocks get a -1e30 triangular mask added on DVE.
  - PV: out^T[d, q] = V.T @ P^T, written to oT unnormalized (bf16).
  - o_proj: out[s, Hc] = O^T.T @ wo_shard, accumulated over the 4 head
    tiles; bf16 output rows merged into 512KB DMA pushes (host sums in f64).
  - scheduling: PE executes strictly in order, so attention's ACT-gated
    stages are emitted interleaved with the next head's QKV chunks and, at
    the tail, with o_proj chains; dummy ident matmuls warm the HAM clock
    gate across DMA-bound windows.

All matmuls run in bf16 (fp32 PSUM accumulation).
"""

import numpy as np
import ml_dtypes

import concourse.bass as bass
import concourse.mybir as mybir
from concourse.tile import TileContext, add_dep_helper
from concourse.vector_clock import ScopedClock
from concourse.masks import make_identity
from concourse._compat import not_none as nn

BF16 = mybir.dt.bfloat16
F32 = mybir.dt.float32
AF = mybir.ActivationFunctionType

B, S, H, NH, NKV, D = 2, 1024, 4096, 32, 8, 128
GROUPS = NH // NKV          # 4 q heads per kv head
S2 = B * S                  # 2048
NKT = H // 128              # 32 k-tiles over H
HPC = NH // 8               # 4 q heads per core
SCALE = float(D) ** -0.5
NEG = -1.0e30
N_CORES = 8
HKT = NKT // 2              # 16 k-tiles per pass
MQ, MK, MV = list(range(HPC)), HPC, HPC + 1   # m indices: q heads, K, V
WCOL = {0: 0, 1: 128, 2: 256, 3: 384, MK: 512, MV: 640}


def _split_multi_waits(nc):
    """The walrus build in this container accepts only ONE sync-wait command
    per instruction. Move extra waits onto same-engine nops inserted just
    before the offending instruction (engine streams execute in block order,
    so waiting at the nop then at the instruction is equivalent)."""
    eng = {
        mybir.EngineType.SP: nc.sync,
        mybir.EngineType.Activation: nc.scalar,
        mybir.EngineType.PE: nc.tensor,
        mybir.EngineType.DVE: nc.vector,
        mybir.EngineType.Pool: nc.gpsimd,
    }
    cur_insts = nn(nc.cur_bb).bb.instructions
    for bb in nc.m.functions[0].blocks:
        insts = bb.instructions
        multi = [i for i in list(insts)
                 if i.sync_info is not None and len(i.sync_info.on_wait or []) > 1]
        for inst in multi:
            ow = list(inst.sync_info.on_wait)
            si = inst.sync_info
            si.on_wait = [ow[-1]]
            inst.sync_info = si
            pos = insts.index(inst)
            for k, w in enumerate(ow[:-1]):
                nop = eng[inst.engine].nop(nofuse=True)
                nop.ins.sync_info = mybir.SyncInfo(on_wait=[w], on_update=[])
                cur_insts.remove(nop.ins)
                insts.insert(pos + k, nop.ins)


class SplitWaitTileContext(TileContext):
    def _drain_and_barrier(self, tick_clock, wait_clock):
        drain_inst = self.nc.sync.drain()
        wait_clock.add_sem_waits(
            drain_inst.ins, ScopedClock({None: tick_clock.global_clock})
        )
        self.nc.all_engine_barrier()
        assert self.sems is not None
        popped = self.nc._tile_sem_poison_stack.pop()
        assert popped is self._sem_poison
        self.nc.clear_and_free_semaphores(list(self.sems.allocated().values()))
        self.nc.all_engine_barrier()
        _split_multi_waits(self.nc)


def _act_reciprocal(nc, out, in_):
    """ACT LUT reciprocal (single pass). bass gates this behind a ValueError
    for accuracy reasons, but HW-measured max rel err here is ~1.2e-5 —
    ample for softmax denominators."""
    eng = nc.scalar
    inputs = [eng.lower_ap(in_)]
    for arg in (0.0, 1.0, 0.0):
        inputs.append(mybir.ImmediateValue(dtype=mybir.dt.float32, value=arg))
    return eng.add_instruction(mybir.InstActivation(
        name=eng.bass.get_next_instruction_name(),
        func=mybir.ActivationFunctionType.Reciprocal,
        ins=inputs, outs=[eng.lower_ap(out)]))


def _attention_blocks(jc):
    """Valid (kt, col-offset, width) S^T blocks for 512-wide q-chunk jc."""
    out = []
    for kt in range(8):
        qlo = 128 * kt           # first valid q for this k-tile (q >= k)
        if qlo < 512 * (jc + 1):
            off = max(0, qlo - 512 * jc)
            out.append((kt, off, 512 - off))
    return out


def build_kernel():
    nc = bass.Bass()

    hT = nc.dram_tensor("hT", [H, S2], BF16, kind="ExternalInput")
    wall = nc.dram_tensor("wall", [NKT, 128, 768], BF16, kind="ExternalInput")
    woh = nc.dram_tensor("woh", [HPC, 128, H], BF16, kind="ExternalInput")
    cosT = nc.dram_tensor("cosT", [128, S2], BF16, kind="ExternalInput")
    sinT = nc.dram_tensor("sinT", [128, S2], BF16, kind="ExternalInput")
    kbias = nc.dram_tensor("kbias", [128, B * 8], F32, kind="ExternalInput")
    seld = nc.dram_tensor("seld", [128, 512], BF16, kind="ExternalInput")
    out = nc.dram_tensor("out", [S2, H], BF16, kind="ExternalOutput")

    with SplitWaitTileContext(nc) as tc:
        with (
            tc.tile_pool(name="const", bufs=1) as cp,
            tc.tile_pool(name="persist", bufs=1) as pp,
        ):
            ident = cp.tile([128, 128], BF16, name="ident")
            make_identity(nc, ident)
            tri = cp.tile([128, 128], F32, name="tri")
            nc.gpsimd.memset(tri, 0.0)
            # keep where j - i >= 0 (upper triangle incl diag); fill NEG below
            nc.gpsimd.affine_select(
                out=tri, in_=tri, compare_op=mybir.AluOpType.is_ge,
                fill=NEG, base=0, pattern=[[1, 128]], channel_multiplier=-1,
            )
            ones_col = cp.tile([128, 1], BF16, name="ones_col")
            nc.gpsimd.memset(ones_col, 1.0)
            # row-selector blocks: sel[:, k*128:(k+1)*128] has ones in row
            # 32k -> sel_k.T @ rhi broadcasts rhi row 32k (rows are spaced 32
            # apart because engine partition bases must be 32-aligned)
            sel = cp.tile([128, 512], BF16, name="sel")
            # persistent denominator pack + reciprocal; initialized to 1.0 so
            # the unused rows stay finite through reciprocal/cast (NaN would
            # poison the selector matmul even when multiplied by 0)
            den_sb = cp.tile([128, 512], F32, name="den_sb")
            nc.gpsimd.memset(den_sb, 1.0)
            rcp = cp.tile([128, 512], F32, name="rcp")
            rhi_t = cp.tile([128, 512], BF16, name="rhi_t")
            kbias_sb = cp.tile([128, B * 8], F32, name="kbias_sb")
            # cos/sin, split in column halves so the first half can arrive
            # early in the load stream (pass-1 ropes its partials directly)
            cos_sb = [cp.tile([128, 1024], BF16, name=f"cos{c}") for c in range(2)]
            sin_sb = [cp.tile([128, 1024], BF16, name=f"sin{c}") for c in range(2)]

            # persistent activations
            qk_roped = [
                pp.tile([128, S2], BF16, name=f"qkr{m}", tag="qkr", bufs=HPC + 1)
                for m in range(HPC + 1)   # 4 q heads + K
            ]
            part_v = pp.tile([128, S2], BF16, name="part_v")
            v_nat = [
                pp.tile([128, 128], BF16, name=f"vn{i}", tag="vnat", bufs=16)
                for i in range(16)
            ]
            oT = [
                pp.tile([128, S2], BF16, name=f"oT{h}", tag="oT", bufs=HPC)
                for h in range(HPC)
            ]

            with tc.tile_pool(name="ph", bufs=1) as ap:
                # ---------------- DMA streams -------------------------------
                # hT is loaded as [128, 1024] column-halves; weights as
                # per-k-tile [128, 768] blocks. Emission order puts the
                # (wall[kt], hT[kt, cols 0:1024]) pairs for pass 1 first so
                # the kt-outer sweep can start ~1.5us in and stay fed.
                hT_sb = {}          # (kt, ch) -> tile
                wc_sb = {}          # kt -> tile
                hT_dmas = []

                def load_h(kt, ch):
                    t = ap.tile([128, 1024], BF16, name=f"hT{kt}_{ch}",
                                tag="hT", bufs=32)
                    d = nc.sync.dma_start(t, hT[kt * 128:(kt + 1) * 128,
                                               ch * 1024:(ch + 1) * 1024])
                    hT_dmas.append(d)
                    hT_sb[(kt, ch)] = t
                    return t

                def load_w(kt):
                    t = ap.tile([128, 768], BF16, name=f"wc{kt}",
                                tag="wc", bufs=19)
                    nc.sync.dma_start(t, wall[kt, :, :])
                    wc_sb[kt] = t
                    return t

                for kt in range(HKT):
                    load_w(kt)
                    load_h(kt, 0)
                    if kt == 10:
                        nc.sync.dma_start(cos_sb[0], cosT[:, 0:1024])
                        nc.sync.dma_start(sin_sb[0], sinT[:, 0:1024])
                nc.sync.dma_start(kbias_sb, kbias[:, :])
                nc.sync.dma_start(sel, seld[:, :])
                for kt in range(HKT):
                    load_h(kt, 1)
                    if kt == 2:
                        nc.sync.dma_start(cos_sb[1], cosT[:, 1024:2048])
                        nc.sync.dma_start(sin_sb[1], sinT[:, 1024:2048])

                # ---------------- rope helpers ------------------------------
                def rope_emit(ps, m, n, first):
                    """qk_roped[m][:, n-chunk] (+)= rope(ps). rope is linear:
                    pass 1 writes rope(partial1); pass 2 adds rope(partial2).
                    ACT evacuates the PSUM partial to bf16 first (releasing the
                    PSUM bank in ~640ns and letting the DVE muls run at bf16
                    rate); the rotate-half swap is folded into the sin mul as
                    two half-muls with shifted output partition base (the sin
                    table is host-rolled by 64 so in0/in1 bases match)."""
                    nsl = slice(n * 512, (n + 1) * 512)
                    csl = slice((n % 2) * 512, (n % 2) * 512 + 512)
                    ch = n // 2
                    qsr = ap.tile([128, 512], BF16, name="qsr", tag="qsr", bufs=3)
                    if first:
                        nc.scalar.copy(qsr, ps)      # ACT is idle in pass 1
                    else:
                        nc.vector.tensor_copy(qsr, ps)  # keep ACT free for exps
                    
                    t1 = ap.tile([128, 512], BF16, name="t1", tag="t1", bufs=2)
                    nc.vector.tensor_mul(t1, qsr, cos_sb[ch][:, csl])
                    t2 = ap.tile([128, 512], BF16, name="t2", tag="t2", bufs=2)
                    nc.vector.tensor_mul(
                        t2[0:64, :], qsr[64:128, :], sin_sb[ch][64:128, csl])
                    nc.vector.tensor_mul(
                        t2[64:128, :], qsr[0:64, :], sin_sb[ch][0:64, csl])
                    dst = qk_roped[m]
                    if first:
                        nc.vector.tensor_add(dst[:, nsl], t1, t2)
                    else:
                        nc.vector.tensor_add(t1, t1, t2)
                        nc.vector.tensor_add(dst[:, nsl], dst[:, nsl], t1)

                def warm_pe(tile, n):
                    """Dummy ident matmuls: free PE activity during DMA-wait
                    windows that lifts the HAM clock gate to 8/8 before the
                    real matmul stream starts (and keeps it there across
                    phase boundaries). start=True on the real accumulation
                    overwrites the junk."""
                    for _ in range(n):
                        nc.tensor.matmul(tile[:, 0:128], ident, ident,
                                         start=True, stop=True)

                # ---------------- pass 1: kt-outer over k-tiles 0..15 -------
                # 6 PSUM accumulators (one per m); four 512-col n-sweeps, the
                # first paced by the hT c0-half DMA stream.
                with tc.tile_pool(name="p1ps", bufs=1, space="PSUM") as ps1:
                    for n in range(4):
                        pss = [
                            ps1.tile([128, 512], F32, name=f"p1_{m}",
                                     tag="p1ps", bufs=6)
                            for m in range(6)
                        ]
                        if n == 0:
                            warm_pe(pss[0], 40)
                        ch, csl = n // 2, slice((n % 2) * 512, (n % 2) * 512 + 512)
                        for kt in range(HKT):
                            for m in range(6):
                                nc.tensor.matmul(
                                    pss[m],
                                    wc_sb[kt][:, WCOL[m]:WCOL[m] + 128],
                                    hT_sb[(kt, ch)][:, csl],
                                    start=(kt == 0), stop=(kt == HKT - 1),
                                )
                        for m in range(6):
                            if m == MV:
                                nc.scalar.copy(
                                    part_v[:, n * 512:(n + 1) * 512], pss[m])
                            else:
                                rope_emit(pss[m], m, n, first=True)

                # second-half streams: weights first, then hT halves (these
                # fill hT/wc pool slots as pass-1 tiles release)
                # h-c0 halves first: their pool slots free during pass-1's
                # n=1 sweep, so these flow early; wc slots only free during the
                # last sweep and would otherwise block the in-order SP queue
                for kt in range(HKT, NKT):
                    load_h(kt, 0)
                for kt in range(HKT, NKT):
                    load_w(kt)
                for kt in range(HKT, NKT):
                    load_h(kt, 1)

                # wo is needed from the o_proj phase on; keep its 4MB off the
                # HBM pipe during the startup-critical loads
                wo_sb = [
                    ap.tile([128, H], BF16, name=f"wo{t}", tag="wo", bufs=HPC)
                    for t in range(HPC)
                ]
                for t in range(HPC):
                    wo_dma = nc.sync.dma_start(wo_sb[t], woh[t, :, :])
                    add_dep_helper(wo_dma.ins, hT_dmas[-1].ins, sync=False,
                                   reason="delay wo load past hidden bulk")

                # ---------------- pass 2 + attention + o_proj ---------------
                # PE executes its stream strictly in order, so attention's
                # ACT-gated stages (denominator after exp) are emitted
                # interleaved with the next head's QKV chunks and, at the
                # tail, with o_proj chains - PE always has independent work
                # queued between dependent stages.
                with tc.tile_pool(name="aps", bufs=1, space="PSUM") as aps:

                    warmed = []

                    def qkv_chunk(pool, m, n, tag="qkvps", bufs=2):
                        ps = pool.tile([128, 512], F32, name="qkvps",
                                       tag=tag, bufs=bufs)
                        if not warmed:
                            warmed.append(1)
                            warm_pe(ps, 12)
                        ch = n // 2
                        csl = slice((n % 2) * 512, (n % 2) * 512 + 512)
                        for kt in range(HKT, NKT):
                            nc.tensor.matmul(
                                ps, wc_sb[kt][:, WCOL[m]:WCOL[m] + 128],
                                hT_sb[(kt, ch)][:, csl],
                                start=(kt == HKT), stop=(kt == NKT - 1),
                            )
                        if m == MV:
                            vsb = ap.tile([128, 512], BF16, name="vsb",
                                          tag="vsb", bufs=1)
                            nc.vector.tensor_add(
                                vsb, ps, part_v[:, n * 512:(n + 1) * 512])
                            for j in range(4):
                                tp = pool.tile([128, 128], BF16, name="tp",
                                               tag="tp", bufs=1)
                                nc.tensor.transpose(
                                    tp, vsb[:, j * 128:(j + 1) * 128], ident)
                                nc.vector.tensor_copy(v_nat[n * 4 + j], tp)
                        else:
                            rope_emit(ps, m, n, first=False)

                    def att_scores(pool, b, h, jc):
                        """Score matmuls + exp issue for one 512-col q chunk.
                        Returns the bf16 P tiles (exp still in flight)."""
                        base = b * S
                        qv = qk_roped[h]
                        kv = qk_roped[MK]
                        p_tiles = {}
                        for kt, off, w in _attention_blocks(jc):
                            qlo = base + 512 * jc + off
                            st = pool.tile([128, 512], F32, name="st",
                                           tag="st", bufs=3)
                            nc.tensor.matmul(
                                st[:, :w],
                                kv[:, base + kt * 128: base + (kt + 1) * 128],
                                qv[:, qlo: qlo + w],
                                start=True, stop=True,
                            )
                            if 128 * kt >= 512 * jc:
                                nc.vector.tensor_add(st[:, :128], st[:, :128], tri)
                            p_sb = ap.tile([128, 512], BF16, name="p_sb",
                                           tag="p", bufs=8)
                            nc.scalar.activation(
                                p_sb[:, :w], st[:, :w], AF.Exp,
                                bias=kbias_sb[:, b * 8 + kt: b * 8 + kt + 1],
                                scale=SCALE,
                            )
                            p_tiles[kt] = p_sb
                        return p_tiles

                    def att_fin_a(pool, b, h, jc, p_tiles, den_sb):
                        """Denominator (evacuated to a packed SBUF row) and PV;
                        the PV result lands in oT UNNORMALIZED (bf16)."""
                        base = b * S
                        blocks = _attention_blocks(jc)
                        dR = pool.tile([128, 512], F32, name="denR",
                                       tag="denR", bufs=1)
                        for i, (kt, off, w) in enumerate(blocks):
                            nc.tensor.matmul(
                                dR[0:1, off:off + w], ones_col,
                                p_tiles[kt][:, :w],
                                start=(i == 0), stop=(i == len(blocks) - 1),
                            )
                        idx = 32 * (b * 2 + jc)
                        nc.vector.tensor_copy(den_sb[idx:idx + 1, :], dR[0:1, :])
                        ot = pool.tile([128, 512], F32, name="ot",
                                       tag="ot", bufs=1)
                        for i, (kt, off, w) in enumerate(blocks):
                            nc.tensor.matmul(
                                ot[:, off:off + w], v_nat[b * 8 + kt],
                                p_tiles[kt][:, :w],
                                start=(i == 0), stop=(i == len(blocks) - 1),
                            )
                        nc.vector.tensor_copy(
                            oT[h][:, base + jc * 512: base + (jc + 1) * 512], ot)

                    def att_recip(den_sb):
                        """One ACT reciprocal for the whole pair (4 rows)."""
                        _act_reciprocal(nc, rcp, den_sb)
                        nc.vector.tensor_copy(rhi_t, rcp)
                        return rhi_t

                    def att_fin_b(pool, b, h, jc, rhi):
                        """Broadcast the reciprocal and scale oT in place."""
                        base = b * S
                        idx = b * 2 + jc
                        bc = pool.tile([128, 512], F32, name="bc",
                                       tag="denR", bufs=1)
                        nc.tensor.matmul(
                            bc, sel[:, idx * 128:(idx + 1) * 128], rhi,
                            start=True, stop=True)
                        osl = oT[h][:, base + jc * 512: base + (jc + 1) * 512]
                        nc.vector.tensor_mul(osl, osl, bc)

                    def att_pair_interleaved(pool, h, fillers, pre=None):
                        """Emit the (b=0, b=1) attention of head h with filler
                        emitters pulled between the ACT-gated stages. Returns
                        a closure finishing the pair's normalization (emitted
                        inside the NEXT slot so its matmuls fill PE gaps)."""
                        stages = [(0, 0), (0, 1), (1, 0), (1, 1)]
                        for i, (b, jc) in enumerate(stages):
                            p = att_scores(pool, b, h, jc)
                            if pre is not None:
                                pre(); pre = None
                            if i < len(fillers):
                                fillers[i]()
                            att_fin_a(pool, b, h, jc, p, den_sb)
                        rhi = att_recip(den_sb)

                        def fin_b(pool2=None):
                            for b, jc in stages:
                                att_fin_b(pool2 if pool2 is not None else pool,
                                          b, h, jc, rhi)
                        return fin_b

                    # slots: qkv of m interleaved with the attention pair of
                    # the previous q head
                    qkv_order = [MK, MV, 0, 1, 2, 3]
                    pending = None
                    for si, m in enumerate(qkv_order):
                        prev_q = m - 1 if m in (1, 2, 3) else None
                        if prev_q is None:
                            for n in range(4):
                                qkv_chunk(aps, m, n)
                        else:
                            nch = 3 if m == 3 else 4
                            pending = att_pair_interleaved(
                                aps, prev_q,
                                [lambda n=n: qkv_chunk(aps, m, n)
                                 for n in range(nch)],
                                pre=pending,
                            )

                # tail: q3's attention; b=0 first (no filler available), then
                # b=1 interleaved with the first o_proj chains of batch 0
                with tc.tile_pool(name="tailps", bufs=1, space="PSUM") as tps:
                    def oproj_half(b, ms, half):
                        base = b * S
                        s0 = base + ms * 128
                        osb = ap.tile([128, 2048], BF16, name="osb",
                                      tag="osb", bufs=2)
                        for j in range(4):
                            nh_ = half * 4 + j
                            po = tps.tile([128, 512], F32, name="po",
                                          tag="po", bufs=3)
                            for ht in range(HPC):
                                nc.tensor.matmul(
                                    po, oT[ht][:, s0:s0 + 128],
                                    wo_sb[ht][:, nh_ * 512:(nh_ + 1) * 512],
                                    start=(ht == 0), stop=(ht == HPC - 1),
                                )
                            if j % 2 == 0:
                                nc.vector.tensor_copy(
                                    osb[:, j * 512:(j + 1) * 512], po)
                            else:
                                nc.scalar.copy(
                                    osb[:, j * 512:(j + 1) * 512], po)
                        nc.sync.dma_start(
                            out[s0:s0 + 128, half * 2048:(half + 1) * 2048], osb
                        )

                    # q3 tail: per-BATCH reciprocal so o_proj b=0 launches as
                    # soon as batch 0's denominators exist, instead of waiting
                    # for batch 1's fin_a through a shared pair reciprocal;
                    # batch 1's ACT-gated stages then ride on o_proj filler
                    p00 = att_scores(tps, 0, 3, 0)
                    if pending is not None:
                        pending(tps)
                    p01 = att_scores(tps, 0, 3, 1)
                    # deferred q3 QKV chunk (cols 1536-2048, first needed by
                    # the batch-1 scores): real PE filler for the exp chain
                    qkv_chunk(tps, 3, 3, tag="po", bufs=3)
                    att_fin_a(tps, 0, 3, 0, p00, den_sb)
                    att_fin_a(tps, 0, 3, 1, p01, den_sb)
                    rhi = att_recip(den_sb)
                    att_fin_b(tps, 0, 3, 0, rhi)
                    att_fin_b(tps, 0, 3, 1, rhi)
                    it = iter([(0, ms, half)
                               for ms in range(8) for half in range(2)])
                    p10 = att_scores(tps, 1, 3, 0)
                    oproj_half(*next(it))
                    oproj_half(*next(it))
                    att_fin_a(tps, 1, 3, 0, p10, den_sb)
                    p11 = att_scores(tps, 1, 3, 1)
                    oproj_half(*next(it))
                    oproj_half(*next(it))
                    att_fin_a(tps, 1, 3, 1, p11, den_sb)
                    rhi = att_recip(den_sb)
                    att_fin_b(tps, 1, 3, 0, rhi)
                    att_fin_b(tps, 1, 3, 1, rhi)
                    for args in it:
                        oproj_half(*args)
                    for ms in range(8):
                        for half in range(2):
                            oproj_half(1, ms, half)
    return nc


_CACHE = {}


def _get_kernel():
    if "nc" not in _CACHE:
        _CACHE["nc"] = build_kernel()
    return _CACHE["nc"]


def _prep_core(c, hT_bf, cosT_bf, sinT_bf, kbias_np, wq, wk, wv, wo):
    bf = ml_dtypes.bfloat16
    # combined per-k-tile weight blocks: [NKT, 128, 768] = q heads | K | V
    sq = wq[:, c * GROUPS * D:(c + 1) * GROUPS * D].reshape(NKT, 128, GROUPS * D)
    sk = wk[:, c * D:(c + 1) * D].reshape(NKT, 128, D)
    sv = wv[:, c * D:(c + 1) * D].reshape(NKT, 128, D)
    wall = np.ascontiguousarray(
        np.concatenate([sq, sk, sv], axis=2)).astype(bf)
    woh = np.ascontiguousarray(
        wo[c * GROUPS * D:(c + 1) * GROUPS * D, :].reshape(HPC, 128, H)
    ).astype(bf)
    # rope halves-swap is applied to (ps * sin) instead of ps, so the sin
    # table itself must be pre-swapped on the partition axis (involution)
    sinT_sw = np.ascontiguousarray(np.roll(sinT_bf, 64, axis=0))
    sel_np = np.zeros((128, 512), ml_dtypes.bfloat16)
    for k in range(4):
        sel_np[32 * k, k * 128:(k + 1) * 128] = 1.0
    return {
        "hT": hT_bf, "wall": wall, "woh": woh,
        "cosT": cosT_bf, "sinT": sinT_sw, "kbias": kbias_np,
        "seld": sel_np,
    }


def kernel(hidden_states, cos, sin, attention_mask, wq, wk, wv, wo):
    from concourse.bass_utils import run_bass_kernel_spmd

    bf = ml_dtypes.bfloat16
    hidden_states = np.asarray(hidden_states, dtype=np.float32)
    cos = np.asarray(cos, dtype=np.float32)
    sin = np.asarray(sin, dtype=np.float32)
    mask = np.asarray(attention_mask)
    wq = np.asarray(wq, dtype=np.float32)
    wk = np.asarray(wk, dtype=np.float32)
    wv = np.asarray(wv, dtype=np.float32)
    wo = np.asarray(wo, dtype=np.float32)

    h2 = hidden_states.reshape(S2, H)
    hT_bf = np.ascontiguousarray(h2.T).astype(bf)
    cosT_bf = np.ascontiguousarray(
        np.concatenate([cos[b].T for b in range(B)], axis=1)
    ).astype(bf)
    ss = sin.copy()
    ss[..., : D // 2] *= -1.0
    sinT_bf = np.ascontiguousarray(
        np.concatenate([ss[b].T for b in range(B)], axis=1)
    ).astype(bf)
    # padding-mask bias, folded into exp's per-partition bias: [128, b*8+kt]
    kbias_np = np.zeros((128, B * 8), np.float32)
    for b in range(B):
        mb = mask[b].astype(bool)
        for kt in range(8):
            kbias_np[:, b * 8 + kt] = np.where(mb[kt * 128:(kt + 1) * 128], 0.0, NEG)
    kbias_np = np.ascontiguousarray(kbias_np)

    in_maps = [
        _prep_core(c, hT_bf, cosT_bf, sinT_bf, kbias_np, wq, wk, wv, wo)
        for c in range(N_CORES)
    ]
    nc = _get_kernel()
    res = run_bass_kernel_spmd(nc, in_maps, core_ids=list(range(N_CORES)))
    acc = np.zeros((S2, H), np.float64)
    for r in res.results:
        acc += r["out"].astype(np.float64)
    return acc.astype(np.float32).reshape(B, S, H)


# revision 32
# speedup vs baseline: 1.0036x; 1.0036x over previous
"""Mixtral attention (B=2, S=1024, H=4096, NH=32, NKV=8, D=128), GQA + RoPE +
causal mask + o_proj, tensor-parallel over heads across 8 TRN2 NeuronCores.

Sharding: core c owns Q heads 4c..4c+3 and KV head c. Each core computes its
heads' attention output and a partial o_proj product (rows of wo owned by its
heads); the host sums the 8 partials.

Layout strategy (per core):
  - host pre-transposes hidden -> hT [H, B*S] so the contraction dim (H) lies
    on SBUF partitions for QKV projections; weights pre-tiled on host into
    per-k-tile combined blocks wall[kt] = [128, 768] (4 q heads | K | V) so
    each arriving hT tile immediately feeds 6 matmuls.
  - QKV projection in two passes over k-halves (only half the hidden is
    SBUF-resident). Pass 1 is kt-outer/m-inner with 6 PSUM accumulators:
    the first n-sweep runs in lockstep with the hT DMA stream (hT loaded in
    column-halves so the first sweep needs only 320KB per k-tile).
  - RoPE is linear, so each pass ropes its partial directly into qk_roped
    (pass 2 adds); no full partial buffers needed for q/K. V keeps a bf16
    partial (part_v) and is transposed to natural [s, d] layout on PE.
  - scores computed transposed: S^T[k, q] = K-block^T.T @ Q^T. Softmax-over-k
    becomes: exp on ACT (scale folded in; padding mask folded into the
    per-partition bias operand), denominator via ones-vector matmul on PE.
    The four denominators of a (b=0,b=1) head pair are packed into rows
    0/32/64/96 of one SBUF tile so a SINGLE ACT reciprocal serves the pair
    (2 ACT table loads per slot instead of per-jc thrash); the reciprocal is
    broadcast to [128, q] by a selector matmul and multiplied into the
    unnormalized PV result in place.
  - causal mask: blocks strictly above the diagonal are skipped outright;
    diagonal 128x128 blocks get a -1e30 triangular mask added on DVE.
  - PV: out^T[d, q] = V.T @ P^T, written to oT unnormalized (bf16).
  - o_proj: out[s, Hc] = O^T.T @ wo_shard, accumulated over the 4 head
    tiles; bf16 output rows merged into 512KB DMA pushes (host sums in f64).
  - scheduling: PE executes strictly in order, so attention's ACT-gated
    stages are emitted interleaved with the next head's QKV chunks and, at
    the tail, with o_proj chains; dummy ident matmuls warm the HAM clock
    gate across DMA-bound windows.

All matmuls run in bf16 (fp32 PSUM accumulation).
"""

import numpy as np
import ml_dtypes

import concourse.bass as bass
import concourse.mybir as mybir
from concourse.tile import TileContext, add_dep_helper
from concourse.vector_clock import ScopedClock
from concourse.masks import make_identity
from concourse._compat import not_none as nn

BF16 = mybir.dt.bfloat16
F32 = mybir.dt.float32
AF = mybir.ActivationFunctionType

B, S, H, NH, NKV, D = 2, 1024, 4096, 32, 8, 128
GROUPS = NH // NKV          # 4 q heads per kv head
S2 = B * S                  # 2048
NKT = H // 128              # 32 k-tiles over H
HPC = NH // 8               # 4 q heads per core
SCALE = float(D) ** -0.5
NEG = -1.0e30
N_CORES = 8
HKT = NKT // 2              # 16 k-tiles per pass
MQ, MK, MV = list(range(HPC)), HPC, HPC + 1   # m indices: q heads, K, V
WCOL = {0: 0, 1: 128, 2: 256, 3: 384, MK: 512, MV: 640}


def _split_multi_waits(nc):
    """The walrus build in this container accepts only ONE sync-wait command
    per instruction. Move extra waits onto same-engine nops inserted just
    before the offending instruction (engine streams execute in block order,
    so waiting at the nop then at the instruction is equivalent)."""
    eng = {
        mybir.EngineType.SP: nc.sync,
        mybir.EngineType.Activation: nc.scalar,
        mybir.EngineType.PE: nc.tensor,
        mybir.EngineType.DVE: nc.vector,
        mybir.EngineType.Pool: nc.gpsimd,
    }
    cur_insts = nn(nc.cur_bb).bb.instructions
    for bb in nc.m.functions[0].blocks:
        insts = bb.instructions
        multi = [i for i in list(insts)
                 if i.sync_info is not None and len(i.sync_info.on_wait or []) > 1]
        for inst in multi:
            ow = list(inst.sync_info.on_wait)
            si = inst.sync_info
            si.on_wait = [ow[-1]]
            inst.sync_info = si
            pos = insts.index(inst)
            for k, w in enumerate(ow[:-1]):
                nop = eng[inst.engine].nop(nofuse=True)
                nop.ins.sync_info = mybir.SyncInfo(on_wait=[w], on_update=[])
                cur_insts.remove(nop.ins)
                insts.insert(pos + k, nop.ins)


class SplitWaitTileContext(TileContext):
    def _drain_and_barrier(self, tick_clock, wait_clock):
        drain_inst = self.nc.sync.drain()
        wait_clock.add_sem_waits(
            drain_inst.ins, ScopedClock({None: tick_clock.global_clock})
        )
        self.nc.all_engine_barrier()
        assert self.sems is not None
        popped = self.nc._tile_sem_poison_stack.pop()
        assert popped is self._sem_poison
        self.nc.clear_and_free_semaphores(list(self.sems.allocated().values()))
        self.nc.all_engine_barrier()
        _split_multi_waits(self.nc)


def _act_reciprocal(nc, out, in_):
    """ACT LUT reciprocal (single pass). bass gates this behind a ValueError
    for accuracy reasons, but HW-measured max rel err here is ~1.2e-5 —
    ample for softmax denominators."""
    eng = nc.scalar
    inputs = [eng.lower_ap(in_)]
    for arg in (0.0, 1.0, 0.0):
        inputs.append(mybir.ImmediateValue(dtype=mybir.dt.float32, value=arg))
    return eng.add_instruction(mybir.InstActivation(
        name=eng.bass.get_next_instruction_name(),
        func=mybir.ActivationFunctionType.Reciprocal,
        ins=inputs, outs=[eng.lower_ap(out)]))


def _attention_blocks(jc):
    """Valid (kt, col-offset, width) S^T blocks for 512-wide q-chunk jc."""
    out = []
    for kt in range(8):
        qlo = 128 * kt           # first valid q for this k-tile (q >= k)
        if qlo < 512 * (jc + 1):
            off = max(0, qlo - 512 * jc)
            out.append((kt, off, 512 - off))
    return out


def build_kernel():
    nc = bass.Bass()

    hT = nc.dram_tensor("hT", [H, S2], BF16, kind="ExternalInput")
    wall = nc.dram_tensor("wall", [NKT, 128, 768], BF16, kind="ExternalInput")
    woh = nc.dram_tensor("woh", [HPC, 128, H], BF16, kind="ExternalInput")
    cosT = nc.dram_tensor("cosT", [128, S2], BF16, kind="ExternalInput")
    sinT = nc.dram_tensor("sinT", [128, S2], BF16, kind="ExternalInput")
    kbias = nc.dram_tensor("kbias", [128, B * 8], F32, kind="ExternalInput")
    seld = nc.dram_tensor("seld", [128, 512], BF16, kind="ExternalInput")
    out = nc.dram_tensor("out", [S2, H], BF16, kind="ExternalOutput")

    with SplitWaitTileContext(nc) as tc:
        with (
            tc.tile_pool(name="const", bufs=1) as cp,
            tc.tile_pool(name="persist", bufs=1) as pp,
        ):
            ident = cp.tile([128, 128], BF16, name="ident")
            make_identity(nc, ident)
            tri = cp.tile([128, 128], F32, name="tri")
            nc.gpsimd.memset(tri, 0.0)
            # keep where j - i >= 0 (upper triangle incl diag); fill NEG below
            nc.gpsimd.affine_select(
                out=tri, in_=tri, compare_op=mybir.AluOpType.is_ge,
                fill=NEG, base=0, pattern=[[1, 128]], channel_multiplier=-1,
            )
            ones_col = cp.tile([128, 1], BF16, name="ones_col")
            nc.gpsimd.memset(ones_col, 1.0)
            # row-selector blocks: sel[:, k*128:(k+1)*128] has ones in row
            # 32k -> sel_k.T @ rhi broadcasts rhi row 32k (rows are spaced 32
            # apart because engine partition bases must be 32-aligned)
            sel = cp.tile([128, 512], BF16, name="sel")
            # persistent denominator pack + reciprocal; initialized to 1.0 so
            # the unused rows stay finite through reciprocal/cast (NaN would
            # poison the selector matmul even when multiplied by 0)
            den_sb = cp.tile([128, 512], F32, name="den_sb")
            nc.gpsimd.memset(den_sb, 1.0)
            rcp = cp.tile([128, 512], F32, name="rcp")
            rhi_t = cp.tile([128, 512], BF16, name="rhi_t")
            kbias_sb = cp.tile([128, B * 8], F32, name="kbias_sb")
            # cos/sin, split in column halves so the first half can arrive
            # early in the load stream (pass-1 ropes its partials directly)
            cos_sb = [cp.tile([128, 1024], BF16, name=f"cos{c}") for c in range(2)]
            sin_sb = [cp.tile([128, 1024], BF16, name=f"sin{c}") for c in range(2)]

            # persistent activations
            qk_roped = [
                pp.tile([128, S2], BF16, name=f"qkr{m}", tag="qkr", bufs=HPC + 1)
                for m in range(HPC + 1)   # 4 q heads + K
            ]
            part_v = pp.tile([128, S2], BF16, name="part_v")
            v_nat = [
                pp.tile([128, 128], BF16, name=f"vn{i}", tag="vnat", bufs=16)
                for i in range(16)
            ]
            oT = [
                pp.tile([128, S2], BF16, name=f"oT{h}", tag="oT", bufs=HPC)
                for h in range(HPC)
            ]

            with tc.tile_pool(name="ph", bufs=1) as ap:
                # ---------------- DMA streams -------------------------------
                # hT is loaded as [128, 1024] column-halves; weights as
                # per-k-tile [128, 768] blocks. Emission order puts the
                # (wall[kt], hT[kt, cols 0:1024]) pairs for pass 1 first so
                # the kt-outer sweep can start ~1.5us in and stay fed.
                hT_sb = {}          # (kt, ch) -> tile
                wc_sb = {}          # kt -> tile
                hT_dmas = []

                def load_h(kt, ch):
                    t = ap.tile([128, 1024], BF16, name=f"hT{kt}_{ch}",
                                tag="hT", bufs=32)
                    d = nc.sync.dma_start(t, hT[kt * 128:(kt + 1) * 128,
                                               ch * 1024:(ch + 1) * 1024])
                    hT_dmas.append(d)
                    hT_sb[(kt, ch)] = t
                    return t

                def load_w(kt):
                    t = ap.tile([128, 768], BF16, name=f"wc{kt}",
                                tag="wc", bufs=19)
                    nc.sync.dma_start(t, wall[kt, :, :])
                    wc_sb[kt] = t
                    return t

                for kt in range(HKT):
                    load_w(kt)
                    load_h(kt, 0)
                    if kt == 10:
                        nc.sync.dma_start(cos_sb[0], cosT[:, 0:1024])
                        nc.sync.dma_start(sin_sb[0], sinT[:, 0:1024])
                nc.sync.dma_start(kbias_sb, kbias[:, :])
                nc.sync.dma_start(sel, seld[:, :])
                for kt in range(HKT):
                    load_h(kt, 1)
                    if kt == 2:
                        nc.sync.dma_start(cos_sb[1], cosT[:, 1024:2048])
                        nc.sync.dma_start(sin_sb[1], sinT[:, 1024:2048])

                # ---------------- rope helpers ------------------------------
                def rope_emit(ps, m, n, first):
                    """qk_roped[m][:, n-chunk] (+)= rope(ps). rope is linear:
                    pass 1 writes rope(partial1); pass 2 adds rope(partial2).
                    ACT evacuates the PSUM partial to bf16 first (releasing the
                    PSUM bank in ~640ns and letting the DVE muls run at bf16
                    rate); the rotate-half swap is folded into the sin mul as
                    two half-muls with shifted output partition base (the sin
                    table is host-rolled by 64 so in0/in1 bases match)."""
                    nsl = slice(n * 512, (n + 1) * 512)
                    csl = slice((n % 2) * 512, (n % 2) * 512 + 512)
                    ch = n // 2
                    qsr = ap.tile([128, 512], BF16, name="qsr", tag="qsr", bufs=3)
                    if first:
                        nc.scalar.copy(qsr, ps)      # ACT is idle in pass 1
                    else:
                        nc.vector.tensor_copy(qsr, ps)  # keep ACT free for exps
                    
                    t1 = ap.tile([128, 512], BF16, name="t1", tag="t1", bufs=2)
                    nc.vector.tensor_mul(t1, qsr, cos_sb[ch][:, csl])
                    t2 = ap.tile([128, 512], BF16, name="t2", tag="t2", bufs=2)
                    nc.vector.tensor_mul(
                        t2[0:64, :], qsr[64:128, :], sin_sb[ch][64:128, csl])
                    nc.vector.tensor_mul(
                        t2[64:128, :], qsr[0:64, :], sin_sb[ch][0:64, csl])
                    dst = qk_roped[m]
                    if first:
                        nc.vector.tensor_add(dst[:, nsl], t1, t2)
                    else:
                        nc.vector.tensor_add(t1, t1, t2)
                        nc.vector.tensor_add(dst[:, nsl], dst[:, nsl], t1)

                def warm_pe(tile, n):
                    """Dummy ident matmuls: free PE activity during DMA-wait
                    windows that lifts the HAM clock gate to 8/8 before the
                    real matmul stream starts (and keeps it there across
                    phase boundaries). start=True on the real accumulation
                    overwrites the junk."""
                    for _ in range(n):
                        nc.tensor.matmul(tile[:, 0:128], ident, ident,
                                         start=True, stop=True)

                # ---------------- pass 1: kt-outer over k-tiles 0..15 -------
                # 6 PSUM accumulators (one per m); four 512-col n-sweeps, the
                # first paced by the hT c0-half DMA stream.
                with tc.tile_pool(name="p1ps", bufs=1, space="PSUM") as ps1:
                    for n in range(4):
                        pss = [
                            ps1.tile([128, 512], F32, name=f"p1_{m}",
                                     tag="p1ps", bufs=6)
                            for m in range(6)
                        ]
                        if n == 0:
                            warm_pe(pss[0], 40)
                        ch, csl = n // 2, slice((n % 2) * 512, (n % 2) * 512 + 512)

                        def evac(m):
                            if m == MV:
                                nc.scalar.copy(
                                    part_v[:, n * 512:(n + 1) * 512], pss[m])
                            else:
                                rope_emit(pss[m], m, n, first=True)

                        if n < 3:
                            # kt-outer: first sweep runs in lockstep with DMA
                            for kt in range(HKT):
                                for m in range(6):
                                    nc.tensor.matmul(
                                        pss[m],
                                        wc_sb[kt][:, WCOL[m]:WCOL[m] + 128],
                                        hT_sb[(kt, ch)][:, csl],
                                        start=(kt == 0), stop=(kt == HKT - 1),
                                    )
                            for m in range(6):
                                evac(m)
                        else:
                            # last sweep m-outer: each accumulator's stop lands
                            # early, so its evacuation (whose PSUM release
                            # gates the pass-2 pool) overlaps the remaining
                            # matmuls instead of draining serially afterwards
                            for m in range(6):
                                for kt in range(HKT):
                                    nc.tensor.matmul(
                                        pss[m],
                                        wc_sb[kt][:, WCOL[m]:WCOL[m] + 128],
                                        hT_sb[(kt, ch)][:, csl],
                                        start=(kt == 0), stop=(kt == HKT - 1),
                                    )
                                evac(m)

                # second-half streams: weights first, then hT halves (these
                # fill hT/wc pool slots as pass-1 tiles release)
                # h-c0 halves first: their pool slots free during pass-1's
                # n=1 sweep, so these flow early; wc slots only free during the
                # last sweep and would otherwise block the in-order SP queue
                for kt in range(HKT, NKT):
                    load_h(kt, 0)
                for kt in range(HKT, NKT):
                    load_w(kt)
                for kt in range(HKT, NKT):
                    load_h(kt, 1)

                # wo is needed from the o_proj phase on; keep its 4MB off the
                # HBM pipe during the startup-critical loads
                wo_sb = [
                    ap.tile([128, H], BF16, name=f"wo{t}", tag="wo", bufs=HPC)
                    for t in range(HPC)
                ]
                for t in range(HPC):
                    wo_dma = nc.sync.dma_start(wo_sb[t], woh[t, :, :])
                    add_dep_helper(wo_dma.ins, hT_dmas[-1].ins, sync=False,
                                   reason="delay wo load past hidden bulk")

                # ---------------- pass 2 + attention + o_proj ---------------
                # PE executes its stream strictly in order, so attention's
                # ACT-gated stages (denominator after exp) are emitted
                # interleaved with the next head's QKV chunks and, at the
                # tail, with o_proj chains - PE always has independent work
                # queued between dependent stages.
                with tc.tile_pool(name="aps", bufs=1, space="PSUM") as aps:

                    warmed = []

                    def qkv_chunk(pool, m, n, tag="qkvps", bufs=2):
                        ps = pool.tile([128, 512], F32, name="qkvps",
                                       tag=tag, bufs=bufs)
                        if not warmed:
                            warmed.append(1)
                            warm_pe(ps, 12)
                        ch = n // 2
                        csl = slice((n % 2) * 512, (n % 2) * 512 + 512)
                        for kt in range(HKT, NKT):
                            nc.tensor.matmul(
                                ps, wc_sb[kt][:, WCOL[m]:WCOL[m] + 128],
                                hT_sb[(kt, ch)][:, csl],
                                start=(kt == HKT), stop=(kt == NKT - 1),
                            )
                        if m == MV:
                            vsb = ap.tile([128, 512], BF16, name="vsb",
                                          tag="vsb", bufs=1)
                            nc.vector.tensor_add(
                                vsb, ps, part_v[:, n * 512:(n + 1) * 512])
                            for j in range(4):
                                tp = pool.tile([128, 128], BF16, name="tp",
                                               tag="tp", bufs=1)
                                nc.tensor.transpose(
                                    tp, vsb[:, j * 128:(j + 1) * 128], ident)
                                nc.vector.tensor_copy(v_nat[n * 4 + j], tp)
                        else:
                            rope_emit(ps, m, n, first=False)

                    def att_scores(pool, b, h, jc):
                        """Score matmuls + exp issue for one 512-col q chunk.
                        Returns the bf16 P tiles (exp still in flight)."""
                        base = b * S
                        qv = qk_roped[h]
                        kv = qk_roped[MK]
                        p_tiles = {}
                        for kt, off, w in _attention_blocks(jc):
                            qlo = base + 512 * jc + off
                            st = pool.tile([128, 512], F32, name="st",
                                           tag="st", bufs=3)
                            nc.tensor.matmul(
                                st[:, :w],
                                kv[:, base + kt * 128: base + (kt + 1) * 128],
                                qv[:, qlo: qlo + w],
                                start=True, stop=True,
                            )
                            if 128 * kt >= 512 * jc:
                                nc.vector.tensor_add(st[:, :128], st[:, :128], tri)
                            p_sb = ap.tile([128, 512], BF16, name="p_sb",
                                           tag="p", bufs=8)
                            nc.scalar.activation(
                                p_sb[:, :w], st[:, :w], AF.Exp,
                                bias=kbias_sb[:, b * 8 + kt: b * 8 + kt + 1],
                                scale=SCALE,
                            )
                            p_tiles[kt] = p_sb
                        return p_tiles

                    def att_fin_a(pool, b, h, jc, p_tiles, den_sb):
                        """Denominator (evacuated to a packed SBUF row) and PV;
                        the PV result lands in oT UNNORMALIZED (bf16)."""
                        base = b * S
                        blocks = _attention_blocks(jc)
                        dR = pool.tile([128, 512], F32, name="denR",
                                       tag="denR", bufs=1)
                        for i, (kt, off, w) in enumerate(blocks):
                            nc.tensor.matmul(
                                dR[0:1, off:off + w], ones_col,
                                p_tiles[kt][:, :w],
                                start=(i == 0), stop=(i == len(blocks) - 1),
                            )
                        idx = 32 * (b * 2 + jc)
                        nc.vector.tensor_copy(den_sb[idx:idx + 1, :], dR[0:1, :])
                        ot = pool.tile([128, 512], F32, name="ot",
                                       tag="ot", bufs=1)
                        for i, (kt, off, w) in enumerate(blocks):
                            nc.tensor.matmul(
                                ot[:, off:off + w], v_nat[b * 8 + kt],
                                p_tiles[kt][:, :w],
                                start=(i == 0), stop=(i == len(blocks) - 1),
                            )
                        nc.vector.tensor_copy(
                            oT[h][:, base + jc * 512: base + (jc + 1) * 512], ot)

                    def att_recip(den_sb):
                        """One ACT reciprocal for the whole pair (4 rows)."""
                        _act_reciprocal(nc, rcp, den_sb)
                        nc.vector.tensor_copy(rhi_t, rcp)
                        return rhi_t

                    def att_fin_b(pool, b, h, jc, rhi):
                        """Broadcast the reciprocal and scale oT in place."""
                        base = b * S
                        idx = b * 2 + jc
                        bc = pool.tile([128, 512], F32, name="bc",
                                       tag="denR", bufs=1)
                        nc.tensor.matmul(
                            bc, sel[:, idx * 128:(idx + 1) * 128], rhi,
                            start=True, stop=True)
                        osl = oT[h][:, base + jc * 512: base + (jc + 1) * 512]
                        nc.vector.tensor_mul(osl, osl, bc)

                    def att_pair_interleaved(pool, h, fillers, pre=None):
                        """Emit the (b=0, b=1) attention of head h with filler
                        emitters pulled between the ACT-gated stages. Returns
                        a closure finishing the pair's normalization (emitted
                        inside the NEXT slot so its matmuls fill PE gaps)."""
                        stages = [(0, 0), (0, 1), (1, 0), (1, 1)]
                        for i, (b, jc) in enumerate(stages):
                            p = att_scores(pool, b, h, jc)
                            if pre is not None:
                                pre(); pre = None
                            if i < len(fillers):
                                fillers[i]()
                            att_fin_a(pool, b, h, jc, p, den_sb)
                        rhi = att_recip(den_sb)

                        def fin_b(pool2=None):
                            for b, jc in stages:
                                att_fin_b(pool2 if pool2 is not None else pool,
                                          b, h, jc, rhi)
                        return fin_b

                    # slots: qkv of m interleaved with the attention pair of
                    # the previous q head
                    qkv_order = [MK, MV, 0, 1, 2, 3]
                    pending = None
                    for si, m in enumerate(qkv_order):
                        prev_q = m - 1 if m in (1, 2, 3) else None
                        if prev_q is None:
                            for n in range(4):
                                qkv_chunk(aps, m, n)
                        else:
                            nch = 3 if m == 3 else 4
                            pending = att_pair_interleaved(
                                aps, prev_q,
                                [lambda n=n: qkv_chunk(aps, m, n)
                                 for n in range(nch)],
                                pre=pending,
                            )

                # tail: q3's attention; b=0 first (no filler available), then
                # b=1 interleaved with the first o_proj chains of batch 0
                with tc.tile_pool(name="tailps", bufs=1, space="PSUM") as tps:
                    def oproj_half(b, ms, half):
                        base = b * S
                        s0 = base + ms * 128
                        osb = ap.tile([128, 2048], BF16, name="osb",
                                      tag="osb", bufs=2)
                        for j in range(4):
                            nh_ = half * 4 + j
                            po = tps.tile([128, 512], F32, name="po",
                                          tag="po", bufs=3)
                            for ht in range(HPC):
                                nc.tensor.matmul(
                                    po, oT[ht][:, s0:s0 + 128],
                                    wo_sb[ht][:, nh_ * 512:(nh_ + 1) * 512],
                                    start=(ht == 0), stop=(ht == HPC - 1),
                                )
                            if j % 2 == 0:
                                nc.vector.tensor_copy(
                                    osb[:, j * 512:(j + 1) * 512], po)
                            else:
                                nc.scalar.copy(
                                    osb[:, j * 512:(j + 1) * 512], po)
                        nc.sync.dma_start(
                            out[s0:s0 + 128, half * 2048:(half + 1) * 2048], osb
                        )

                    # q3 tail: per-BATCH reciprocal so o_proj b=0 launches as
                    # soon as batch 0's denominators exist, instead of waiting
                    # for batch 1's fin_a through a shared pair reciprocal;
                    # batch 1's ACT-gated stages then ride on o_proj filler
                    p00 = att_scores(tps, 0, 3, 0)
                    if pending is not None:
                        pending(tps)
                    p01 = att_scores(tps, 0, 3, 1)
                    # deferred q3 QKV chunk (cols 1536-2048, first needed by
                    # the batch-1 scores): real PE filler for the exp chain
                    qkv_chunk(tps, 3, 3, tag="po", bufs=3)
                    att_fin_a(tps, 0, 3, 0, p00, den_sb)
                    att_fin_a(tps, 0, 3, 1, p01, den_sb)
                    rhi = att_recip(den_sb)
                    att_fin_b(tps, 0, 3, 0, rhi)
                    att_fin_b(tps, 0, 3, 1, rhi)
                    it = iter([(0, ms, half)
                               for ms in range(8) for half in range(2)])
                    p10 = att_scores(tps, 1, 3, 0)
                    oproj_half(*next(it))
                    oproj_half(*next(it))
                    att_fin_a(tps, 1, 3, 0, p10, den_sb)
                    p11 = att_scores(tps, 1, 3, 1)
                    oproj_half(*next(it))
                    oproj_half(*next(it))
                    att_fin_a(tps, 1, 3, 1, p11, den_sb)
                    rhi = att_recip(den_sb)
                    att_fin_b(tps, 1, 3, 0, rhi)
                    att_fin_b(tps, 1, 3, 1, rhi)
                    for args in it:
                        oproj_half(*args)
                    for ms in range(8):
                        for half in range(2):
                            oproj_half(1, ms, half)
    return nc


_CACHE = {}


def _get_kernel():
    if "nc" not in _CACHE:
        _CACHE["nc"] = build_kernel()
    return _CACHE["nc"]


def _prep_core(c, hT_bf, cosT_bf, sinT_bf, kbias_np, wq, wk, wv, wo):
    bf = ml_dtypes.bfloat16
    # combined per-k-tile weight blocks: [NKT, 128, 768] = q heads | K | V
    sq = wq[:, c * GROUPS * D:(c + 1) * GROUPS * D].reshape(NKT, 128, GROUPS * D)
    sk = wk[:, c * D:(c + 1) * D].reshape(NKT, 128, D)
    sv = wv[:, c * D:(c + 1) * D].reshape(NKT, 128, D)
    wall = np.ascontiguousarray(
        np.concatenate([sq, sk, sv], axis=2)).astype(bf)
    woh = np.ascontiguousarray(
        wo[c * GROUPS * D:(c + 1) * GROUPS * D, :].reshape(HPC, 128, H)
    ).astype(bf)
    # rope halves-swap is applied to (ps * sin) instead of ps, so the sin
    # table itself must be pre-swapped on the partition axis (involution)
    sinT_sw = np.ascontiguousarray(np.roll(sinT_bf, 64, axis=0))
    sel_np = np.zeros((128, 512), ml_dtypes.bfloat16)
    for k in range(4):
        sel_np[32 * k, k * 128:(k + 1) * 128] = 1.0
    return {
        "hT": hT_bf, "wall": wall, "woh": woh,
        "cosT": cosT_bf, "sinT": sinT_sw, "kbias": kbias_np,
        "seld": sel_np,
    }


def kernel(hidden_states, cos, sin, attention_mask, wq, wk, wv, wo):
    from concourse.bass_utils import run_bass_kernel_spmd

    bf = ml_dtypes.bfloat16
    hidden_states = np.asarray(hidden_states, dtype=np.float32)
    cos = np.asarray(cos, dtype=np.float32)
    sin = np.asarray(sin, dtype=np.float32)
    mask = np.asarray(attention_mask)
    wq = np.asarray(wq, dtype=np.float32)
    wk = np.asarray(wk, dtype=np.float32)
    wv = np.asarray(wv, dtype=np.float32)
    wo = np.asarray(wo, dtype=np.float32)

    h2 = hidden_states.reshape(S2, H)
    hT_bf = np.ascontiguousarray(h2.T).astype(bf)
    cosT_bf = np.ascontiguousarray(
        np.concatenate([cos[b].T for b in range(B)], axis=1)
    ).astype(bf)
    ss = sin.copy()
    ss[..., : D // 2] *= -1.0
    sinT_bf = np.ascontiguousarray(
        np.concatenate([ss[b].T for b in range(B)], axis=1)
    ).astype(bf)
    # padding-mask bias, folded into exp's per-partition bias: [128, b*8+kt]
    kbias_np = np.zeros((128, B * 8), np.float32)
    for b in range(B):
        mb = mask[b].astype(bool)
        for kt in range(8):
            kbias_np[:, b * 8 + kt] = np.where(mb[kt * 128:(kt + 1) * 128], 0.0, NEG)
    kbias_np = np.ascontiguousarray(kbias_np)

    in_maps = [
        _prep_core(c, hT_bf, cosT_bf, sinT_bf, kbias_np, wq, wk, wv, wo)
        for c in range(N_CORES)
    ]
    nc = _get_kernel()
    res = run_bass_kernel_spmd(nc, in_maps, core_ids=list(range(N_CORES)))
    acc = np.zeros((S2, H), np.float64)
    for r in res.results:
        acc += r["out"].astype(np.float64)
    return acc.astype(np.float32).reshape(B, S, H)


# revision 33
# speedup vs baseline: 1.0067x; 1.0031x over previous
"""Mixtral attention (B=2, S=1024, H=4096, NH=32, NKV=8, D=128), GQA + RoPE +
causal mask + o_proj, tensor-parallel over heads across 8 TRN2 NeuronCores.

Sharding: core c owns Q heads 4c..4c+3 and KV head c. Each core computes its
heads' attention output and a partial o_proj product (rows of wo owned by its
heads); the host sums the 8 partials.

Layout strategy (per core):
  - host pre-transposes hidden -> hT [H, B*S] so the contraction dim (H) lies
    on SBUF partitions for QKV projections; weights pre-tiled on host into
    per-k-tile combined blocks wall[kt] = [128, 768] (4 q heads | K | V) so
    each arriving hT tile immediately feeds 6 matmuls.
  - QKV projection in two passes over k-halves (only half the hidden is
    SBUF-resident). Pass 1 is kt-outer/m-inner with 6 PSUM accumulators:
    the first n-sweep runs in lockstep with the hT DMA stream (hT loaded in
    column-halves so the first sweep needs only 320KB per k-tile).
  - RoPE is linear, so each pass ropes its partial directly into qk_roped
    (pass 2 adds); no full partial buffers needed for q/K. V keeps a bf16
    partial (part_v) and is transposed to natural [s, d] layout on PE.
  - scores computed transposed: S^T[k, q] = K-block^T.T @ Q^T. Softmax-over-k
    becomes: exp on ACT (scale folded in; padding mask folded into the
    per-partition bias operand), denominator via ones-vector matmul on PE.
    The four denominators of a (b=0,b=1) head pair are packed into rows
    0/32/64/96 of one SBUF tile so a SINGLE ACT reciprocal serves the pair
    (2 ACT table loads per slot instead of per-jc thrash); the reciprocal is
    broadcast to [128, q] by a selector matmul and multiplied into the
    unnormalized PV result in place.
  - causal mask: blocks strictly above the diagonal are skipped outright;
    diagonal 128x128 blocks get a -1e30 triangular mask added on DVE.
  - PV: out^T[d, q] = V.T @ P^T, written to oT unnormalized (bf16).
  - o_proj: out[s, Hc] = O^T.T @ wo_shard, accumulated over the 4 head
    tiles; bf16 output rows merged into 512KB DMA pushes (host sums in f64).
  - scheduling: PE executes strictly in order, so attention's ACT-gated
    stages are emitted interleaved with the next head's QKV chunks and, at
    the tail, with o_proj chains; dummy ident matmuls warm the HAM clock
    gate across DMA-bound windows.

All matmuls run in bf16 (fp32 PSUM accumulation).
"""

import numpy as np
import ml_dtypes

import concourse.bass as bass
import concourse.mybir as mybir
from concourse.tile import TileContext, add_dep_helper
from concourse.vector_clock import ScopedClock
from concourse.masks import make_identity
from concourse._compat import not_none as nn

BF16 = mybir.dt.bfloat16
F32 = mybir.dt.float32
AF = mybir.ActivationFunctionType

B, S, H, NH, NKV, D = 2, 1024, 4096, 32, 8, 128
GROUPS = NH // NKV          # 4 q heads per kv head
S2 = B * S                  # 2048
NKT = H // 128              # 32 k-tiles over H
HPC = NH // 8               # 4 q heads per core
SCALE = float(D) ** -0.5
NEG = -1.0e30
N_CORES = 8
HKT = NKT // 2              # 16 k-tiles per pass
MQ, MK, MV = list(range(HPC)), HPC, HPC + 1   # m indices: q heads, K, V
WCOL = {0: 0, 1: 128, 2: 256, 3: 384, MK: 512, MV: 640}


def _split_multi_waits(nc):
    """The walrus build in this container accepts only ONE sync-wait command
    per instruction. Move extra waits onto same-engine nops inserted just
    before the offending instruction (engine streams execute in block order,
    so waiting at the nop then at the instruction is equivalent)."""
    eng = {
        mybir.EngineType.SP: nc.sync,
        mybir.EngineType.Activation: nc.scalar,
        mybir.EngineType.PE: nc.tensor,
        mybir.EngineType.DVE: nc.vector,
        mybir.EngineType.Pool: nc.gpsimd,
    }
    cur_insts = nn(nc.cur_bb).bb.instructions
    for bb in nc.m.functions[0].blocks:
        insts = bb.instructions
        multi = [i for i in list(insts)
                 if i.sync_info is not None and len(i.sync_info.on_wait or []) > 1]
        for inst in multi:
            ow = list(inst.sync_info.on_wait)
            si = inst.sync_info
            si.on_wait = [ow[-1]]
            inst.sync_info = si
            pos = insts.index(inst)
            for k, w in enumerate(ow[:-1]):
                nop = eng[inst.engine].nop(nofuse=True)
                nop.ins.sync_info = mybir.SyncInfo(on_wait=[w], on_update=[])
                cur_insts.remove(nop.ins)
                insts.insert(pos + k, nop.ins)


class SplitWaitTileContext(TileContext):
    def _drain_and_barrier(self, tick_clock, wait_clock):
        drain_inst = self.nc.sync.drain()
        wait_clock.add_sem_waits(
            drain_inst.ins, ScopedClock({None: tick_clock.global_clock})
        )
        self.nc.all_engine_barrier()
        assert self.sems is not None
        popped = self.nc._tile_sem_poison_stack.pop()
        assert popped is self._sem_poison
        self.nc.clear_and_free_semaphores(list(self.sems.allocated().values()))
        self.nc.all_engine_barrier()
        _split_multi_waits(self.nc)


def _act_reciprocal(nc, out, in_):
    """ACT LUT reciprocal (single pass). bass gates this behind a ValueError
    for accuracy reasons, but HW-measured max rel err here is ~1.2e-5 —
    ample for softmax denominators."""
    eng = nc.scalar
    inputs = [eng.lower_ap(in_)]
    for arg in (0.0, 1.0, 0.0):
        inputs.append(mybir.ImmediateValue(dtype=mybir.dt.float32, value=arg))
    return eng.add_instruction(mybir.InstActivation(
        name=eng.bass.get_next_instruction_name(),
        func=mybir.ActivationFunctionType.Reciprocal,
        ins=inputs, outs=[eng.lower_ap(out)]))


def _attention_blocks(jc):
    """Valid (kt, col-offset, width) S^T blocks for 512-wide q-chunk jc."""
    out = []
    for kt in range(8):
        qlo = 128 * kt           # first valid q for this k-tile (q >= k)
        if qlo < 512 * (jc + 1):
            off = max(0, qlo - 512 * jc)
            out.append((kt, off, 512 - off))
    return out


def build_kernel():
    nc = bass.Bass()

    hT = nc.dram_tensor("hT", [H, S2], BF16, kind="ExternalInput")
    wall = nc.dram_tensor("wall", [NKT, 128, 768], BF16, kind="ExternalInput")
    woh = nc.dram_tensor("woh", [HPC, 128, H], BF16, kind="ExternalInput")
    cosT = nc.dram_tensor("cosT", [128, S2], BF16, kind="ExternalInput")
    sinT = nc.dram_tensor("sinT", [128, S2], BF16, kind="ExternalInput")
    kbias = nc.dram_tensor("kbias", [128, B * 8], F32, kind="ExternalInput")
    seld = nc.dram_tensor("seld", [128, 512], BF16, kind="ExternalInput")
    out = nc.dram_tensor("out", [S2, H], BF16, kind="ExternalOutput")

    with SplitWaitTileContext(nc) as tc:
        with (
            tc.tile_pool(name="const", bufs=1) as cp,
            tc.tile_pool(name="persist", bufs=1) as pp,
        ):
            ident = cp.tile([128, 128], BF16, name="ident")
            make_identity(nc, ident)
            tri = cp.tile([128, 128], F32, name="tri")
            nc.gpsimd.memset(tri, 0.0)
            # keep where j - i >= 0 (upper triangle incl diag); fill NEG below
            nc.gpsimd.affine_select(
                out=tri, in_=tri, compare_op=mybir.AluOpType.is_ge,
                fill=NEG, base=0, pattern=[[1, 128]], channel_multiplier=-1,
            )
            ones_col = cp.tile([128, 1], BF16, name="ones_col")
            nc.gpsimd.memset(ones_col, 1.0)
            # row-selector blocks: sel[:, k*128:(k+1)*128] has ones in row
            # 32k -> sel_k.T @ rhi broadcasts rhi row 32k (rows are spaced 32
            # apart because engine partition bases must be 32-aligned)
            sel = cp.tile([128, 512], BF16, name="sel")
            # persistent denominator pack + reciprocal; initialized to 1.0 so
            # the unused rows stay finite through reciprocal/cast (NaN would
            # poison the selector matmul even when multiplied by 0)
            den_sb = cp.tile([128, 512], F32, name="den_sb")
            nc.gpsimd.memset(den_sb, 1.0)
            rcp = cp.tile([128, 512], F32, name="rcp")
            rhi_t = cp.tile([128, 512], BF16, name="rhi_t")
            kbias_sb = cp.tile([128, B * 8], F32, name="kbias_sb")
            # cos/sin, split in column halves so the first half can arrive
            # early in the load stream (pass-1 ropes its partials directly)
            cos_sb = [cp.tile([128, 1024], BF16, name=f"cos{c}") for c in range(2)]
            sin_sb = [cp.tile([128, 1024], BF16, name=f"sin{c}") for c in range(2)]

            # persistent activations
            qk_roped = [
                pp.tile([128, S2], BF16, name=f"qkr{m}", tag="qkr", bufs=HPC + 1)
                for m in range(HPC + 1)   # 4 q heads + K
            ]
            part_v = pp.tile([128, S2], BF16, name="part_v")
            v_nat = [
                pp.tile([128, 128], BF16, name=f"vn{i}", tag="vnat", bufs=16)
                for i in range(16)
            ]
            oT = [
                pp.tile([128, S2], BF16, name=f"oT{h}", tag="oT", bufs=HPC)
                for h in range(HPC)
            ]

            with tc.tile_pool(name="ph", bufs=1) as ap:
                # ---------------- DMA streams -------------------------------
                # hT is loaded as [128, 1024] column-halves; weights as
                # per-k-tile [128, 768] blocks. Emission order puts the
                # (wall[kt], hT[kt, cols 0:1024]) pairs for pass 1 first so
                # the kt-outer sweep can start ~1.5us in and stay fed.
                hT_sb = {}          # (kt, ch) -> tile
                wc_sb = {}          # kt -> tile
                hT_dmas = []

                def load_h(kt, ch):
                    t = ap.tile([128, 1024], BF16, name=f"hT{kt}_{ch}",
                                tag="hT", bufs=32)
                    d = nc.sync.dma_start(t, hT[kt * 128:(kt + 1) * 128,
                                               ch * 1024:(ch + 1) * 1024])
                    hT_dmas.append(d)
                    hT_sb[(kt, ch)] = t
                    return t

                def load_w(kt):
                    t = ap.tile([128, 768], BF16, name=f"wc{kt}",
                                tag="wc", bufs=19)
                    nc.sync.dma_start(t, wall[kt, :, :])
                    wc_sb[kt] = t
                    return t

                for kt in range(HKT):
                    load_w(kt)
                    load_h(kt, 0)
                    if kt == 10:
                        nc.sync.dma_start(cos_sb[0], cosT[:, 0:1024])
                        nc.sync.dma_start(sin_sb[0], sinT[:, 0:1024])
                nc.sync.dma_start(kbias_sb, kbias[:, :])
                nc.sync.dma_start(sel, seld[:, :])
                for kt in range(HKT):
                    load_h(kt, 1)
                    if kt == 2:
                        nc.sync.dma_start(cos_sb[1], cosT[:, 1024:2048])
                        nc.sync.dma_start(sin_sb[1], sinT[:, 1024:2048])

                # ---------------- rope helpers ------------------------------
                def rope_emit(ps, m, n, first):
                    """qk_roped[m][:, n-chunk] (+)= rope(ps). rope is linear:
                    pass 1 writes rope(partial1); pass 2 adds rope(partial2).
                    ACT evacuates the PSUM partial to bf16 first (releasing the
                    PSUM bank in ~640ns and letting the DVE muls run at bf16
                    rate); the rotate-half swap is folded into the sin mul as
                    two half-muls with shifted output partition base (the sin
                    table is host-rolled by 64 so in0/in1 bases match)."""
                    nsl = slice(n * 512, (n + 1) * 512)
                    csl = slice((n % 2) * 512, (n % 2) * 512 + 512)
                    ch = n // 2
                    qsr = ap.tile([128, 512], BF16, name="qsr", tag="qsr", bufs=3)
                    if first:
                        nc.scalar.copy(qsr, ps)      # ACT is idle in pass 1
                    else:
                        nc.vector.tensor_copy(qsr, ps)  # keep ACT free for exps
                    
                    t1 = ap.tile([128, 512], BF16, name="t1", tag="t1", bufs=2)
                    nc.vector.tensor_mul(t1, qsr, cos_sb[ch][:, csl])
                    t2 = ap.tile([128, 512], BF16, name="t2", tag="t2", bufs=2)
                    nc.vector.tensor_mul(
                        t2[0:64, :], qsr[64:128, :], sin_sb[ch][64:128, csl])
                    nc.vector.tensor_mul(
                        t2[64:128, :], qsr[0:64, :], sin_sb[ch][0:64, csl])
                    dst = qk_roped[m]
                    if first:
                        nc.vector.tensor_add(dst[:, nsl], t1, t2)
                    else:
                        nc.vector.tensor_add(t1, t1, t2)
                        nc.vector.tensor_add(dst[:, nsl], dst[:, nsl], t1)

                def warm_pe(tile, n):
                    """Dummy ident matmuls: free PE activity during DMA-wait
                    windows that lifts the HAM clock gate to 8/8 before the
                    real matmul stream starts (and keeps it there across
                    phase boundaries). start=True on the real accumulation
                    overwrites the junk."""
                    for _ in range(n):
                        nc.tensor.matmul(tile[:, 0:128], ident, ident,
                                         start=True, stop=True)

                # ---------------- pass 1: kt-outer over k-tiles 0..15 -------
                # 6 PSUM accumulators (one per m); four 512-col n-sweeps, the
                # first paced by the hT c0-half DMA stream.
                with tc.tile_pool(name="p1ps", bufs=1, space="PSUM") as ps1:
                    for n in range(4):
                        pss = [
                            ps1.tile([128, 512], F32, name=f"p1_{m}",
                                     tag="p1ps", bufs=6)
                            for m in range(6)
                        ]
                        if n == 0:
                            warm_pe(pss[0], 40)
                        ch, csl = n // 2, slice((n % 2) * 512, (n % 2) * 512 + 512)

                        def evac(m):
                            if m == MV:
                                nc.scalar.copy(
                                    part_v[:, n * 512:(n + 1) * 512], pss[m])
                            else:
                                rope_emit(pss[m], m, n, first=True)

                        for kt in range(HKT):
                            for m in range(6):
                                nc.tensor.matmul(
                                    pss[m],
                                    wc_sb[kt][:, WCOL[m]:WCOL[m] + 128],
                                    hT_sb[(kt, ch)][:, csl],
                                    start=(kt == 0), stop=(kt == HKT - 1),
                                )
                        for m in range(6):
                            evac(m)

                # second-half streams: weights first, then hT halves (these
                # fill hT/wc pool slots as pass-1 tiles release)
                # h-c0 halves first: their pool slots free during pass-1's
                # n=1 sweep, so these flow early; wc slots only free during the
                # last sweep and would otherwise block the in-order SP queue
                for kt in range(HKT, NKT):
                    load_h(kt, 0)
                for kt in range(HKT, NKT):
                    load_w(kt)
                for kt in range(HKT, NKT):
                    load_h(kt, 1)

                # wo is needed from the o_proj phase on; keep its 4MB off the
                # HBM pipe during the startup-critical loads
                wo_sb = [
                    ap.tile([128, H], BF16, name=f"wo{t}", tag="wo", bufs=HPC)
                    for t in range(HPC)
                ]
                for t in range(HPC):
                    wo_dma = nc.sync.dma_start(wo_sb[t], woh[t, :, :])
                    add_dep_helper(wo_dma.ins, hT_dmas[-1].ins, sync=False,
                                   reason="delay wo load past hidden bulk")

                # ---------------- pass 2 + attention + o_proj ---------------
                # PE executes its stream strictly in order, so attention's
                # ACT-gated stages (denominator after exp) are emitted
                # interleaved with the next head's QKV chunks and, at the
                # tail, with o_proj chains - PE always has independent work
                # queued between dependent stages.
                with tc.tile_pool(name="aps", bufs=1, space="PSUM") as aps:

                    warmed = []

                    def qkv_chunk(pool, m, n, tag="qkvps", bufs=2):
                        ps = pool.tile([128, 512], F32, name="qkvps",
                                       tag=tag, bufs=bufs)
                        if not warmed:
                            warmed.append(1)
                            warm_pe(ps, 12)
                        ch = n // 2
                        csl = slice((n % 2) * 512, (n % 2) * 512 + 512)
                        for kt in range(HKT, NKT):
                            nc.tensor.matmul(
                                ps, wc_sb[kt][:, WCOL[m]:WCOL[m] + 128],
                                hT_sb[(kt, ch)][:, csl],
                                start=(kt == HKT), stop=(kt == NKT - 1),
                            )
                        if m == MV:
                            vsb = ap.tile([128, 512], BF16, name="vsb",
                                          tag="vsb", bufs=1)
                            nc.vector.tensor_add(
                                vsb, ps, part_v[:, n * 512:(n + 1) * 512])
                            for j in range(4):
                                tp = pool.tile([128, 128], BF16, name="tp",
                                               tag="tp", bufs=1)
                                nc.tensor.transpose(
                                    tp, vsb[:, j * 128:(j + 1) * 128], ident)
                                nc.vector.tensor_copy(v_nat[n * 4 + j], tp)
                        else:
                            rope_emit(ps, m, n, first=False)

                    def att_scores(pool, b, h, jc):
                        """Score matmuls + exp issue for one 512-col q chunk.
                        Returns the bf16 P tiles (exp still in flight)."""
                        base = b * S
                        qv = qk_roped[h]
                        kv = qk_roped[MK]
                        p_tiles = {}
                        for kt, off, w in _attention_blocks(jc):
                            qlo = base + 512 * jc + off
                            st = pool.tile([128, 512], F32, name="st",
                                           tag="st", bufs=3)
                            nc.tensor.matmul(
                                st[:, :w],
                                kv[:, base + kt * 128: base + (kt + 1) * 128],
                                qv[:, qlo: qlo + w],
                                start=True, stop=True,
                            )
                            if 128 * kt >= 512 * jc:
                                nc.vector.tensor_add(st[:, :128], st[:, :128], tri)
                            p_sb = ap.tile([128, 512], BF16, name="p_sb",
                                           tag="p", bufs=8)
                            nc.scalar.activation(
                                p_sb[:, :w], st[:, :w], AF.Exp,
                                bias=kbias_sb[:, b * 8 + kt: b * 8 + kt + 1],
                                scale=SCALE,
                            )
                            p_tiles[kt] = p_sb
                        return p_tiles

                    def att_fin_a(pool, b, h, jc, p_tiles, den_sb):
                        """Denominator (evacuated to a packed SBUF row) and PV;
                        the PV result lands in oT UNNORMALIZED (bf16)."""
                        base = b * S
                        blocks = _attention_blocks(jc)
                        dR = pool.tile([128, 512], F32, name="denR",
                                       tag="denR", bufs=1)
                        for i, (kt, off, w) in enumerate(blocks):
                            nc.tensor.matmul(
                                dR[0:1, off:off + w], ones_col,
                                p_tiles[kt][:, :w],
                                start=(i == 0), stop=(i == len(blocks) - 1),
                            )
                        idx = 32 * (b * 2 + jc)
                        nc.vector.tensor_copy(den_sb[idx:idx + 1, :], dR[0:1, :])
                        ot = pool.tile([128, 512], F32, name="ot",
                                       tag="ot", bufs=1)
                        for i, (kt, off, w) in enumerate(blocks):
                            nc.tensor.matmul(
                                ot[:, off:off + w], v_nat[b * 8 + kt],
                                p_tiles[kt][:, :w],
                                start=(i == 0), stop=(i == len(blocks) - 1),
                            )
                        nc.vector.tensor_copy(
                            oT[h][:, base + jc * 512: base + (jc + 1) * 512], ot)

                    def att_recip(den_sb):
                        """One ACT reciprocal for the whole pair (4 rows)."""
                        _act_reciprocal(nc, rcp, den_sb)
                        nc.vector.tensor_copy(rhi_t, rcp)
                        return rhi_t

                    def att_fin_b(pool, b, h, jc, rhi):
                        """Broadcast the reciprocal and scale oT in place."""
                        base = b * S
                        idx = b * 2 + jc
                        bc = pool.tile([128, 512], F32, name="bc",
                                       tag="denR", bufs=1)
                        nc.tensor.matmul(
                            bc, sel[:, idx * 128:(idx + 1) * 128], rhi,
                            start=True, stop=True)
                        osl = oT[h][:, base + jc * 512: base + (jc + 1) * 512]
                        nc.vector.tensor_mul(osl, osl, bc)

                    def att_pair_interleaved(pool, h, fillers, pre=None):
                        """Emit the (b=0, b=1) attention of head h with filler
                        emitters pulled between the ACT-gated stages. Returns
                        a closure finishing the pair's normalization (emitted
                        inside the NEXT slot so its matmuls fill PE gaps)."""
                        stages = [(0, 0), (0, 1), (1, 0), (1, 1)]
                        for i, (b, jc) in enumerate(stages):
                            p = att_scores(pool, b, h, jc)
                            if pre is not None:
                                pre(); pre = None
                            if i < len(fillers):
                                fillers[i]()
                            att_fin_a(pool, b, h, jc, p, den_sb)
                        rhi = att_recip(den_sb)

                        def fin_b(pool2=None):
                            for b, jc in stages:
                                att_fin_b(pool2 if pool2 is not None else pool,
                                          b, h, jc, rhi)
                        return fin_b

                    # slots: qkv of m interleaved with the attention pair of
                    # the previous q head
                    qkv_order = [MK, MV, 0, 1, 2, 3]
                    pending = None
                    for si, m in enumerate(qkv_order):
                        prev_q = m - 1 if m in (1, 2, 3) else None
                        if prev_q is None:
                            for n in range(4):
                                qkv_chunk(aps, m, n)
                        else:
                            nch = 3 if m == 3 else 4
                            pending = att_pair_interleaved(
                                aps, prev_q,
                                [lambda n=n: qkv_chunk(aps, m, n)
                                 for n in range(nch)],
                                pre=pending,
                            )

                # tail: q3's attention; b=0 first (no filler available), then
                # b=1 interleaved with the first o_proj chains of batch 0
                with tc.tile_pool(name="tailps", bufs=1, space="PSUM") as tps:
                    def oproj_half(b, ms, half):
                        base = b * S
                        s0 = base + ms * 128
                        osb = ap.tile([128, 2048], BF16, name="osb",
                                      tag="osb", bufs=2)
                        for j in range(4):
                            nh_ = half * 4 + j
                            po = tps.tile([128, 512], F32, name="po",
                                          tag="po", bufs=3)
                            for ht in range(HPC):
                                nc.tensor.matmul(
                                    po, oT[ht][:, s0:s0 + 128],
                                    wo_sb[ht][:, nh_ * 512:(nh_ + 1) * 512],
                                    start=(ht == 0), stop=(ht == HPC - 1),
                                )
                            if j % 2 == 0:
                                nc.vector.tensor_copy(
                                    osb[:, j * 512:(j + 1) * 512], po)
                            else:
                                nc.scalar.copy(
                                    osb[:, j * 512:(j + 1) * 512], po)
                        nc.sync.dma_start(
                            out[s0:s0 + 128, half * 2048:(half + 1) * 2048], osb
                        )

                    # q3 tail: per-BATCH reciprocal so o_proj b=0 launches as
                    # soon as batch 0's denominators exist, instead of waiting
                    # for batch 1's fin_a through a shared pair reciprocal;
                    # batch 1's ACT-gated stages then ride on o_proj filler
                    p00 = att_scores(tps, 0, 3, 0)
                    if pending is not None:
                        pending(tps)
                    p01 = att_scores(tps, 0, 3, 1)
                    # deferred q3 QKV chunk (cols 1536-2048, first needed by
                    # the batch-1 scores): real PE filler for the exp chain
                    qkv_chunk(tps, 3, 3, tag="po", bufs=3)
                    att_fin_a(tps, 0, 3, 0, p00, den_sb)
                    att_fin_a(tps, 0, 3, 1, p01, den_sb)
                    rhi = att_recip(den_sb)
                    att_fin_b(tps, 0, 3, 0, rhi)
                    att_fin_b(tps, 0, 3, 1, rhi)
                    it = iter([(0, ms, half)
                               for ms in range(8) for half in range(2)])
                    p10 = att_scores(tps, 1, 3, 0)
                    oproj_half(*next(it))
                    oproj_half(*next(it))
                    att_fin_a(tps, 1, 3, 0, p10, den_sb)
                    p11 = att_scores(tps, 1, 3, 1)
                    oproj_half(*next(it))
                    oproj_half(*next(it))
                    att_fin_a(tps, 1, 3, 1, p11, den_sb)
                    rhi = att_recip(den_sb)
                    att_fin_b(tps, 1, 3, 0, rhi)
                    att_fin_b(tps, 1, 3, 1, rhi)
                    for args in it:
                        oproj_half(*args)
                    for ms in range(8):
                        for half in range(2):
                            oproj_half(1, ms, half)
    return nc


_CACHE = {}


def _get_kernel():
    if "nc" not in _CACHE:
        _CACHE["nc"] = build_kernel()
    return _CACHE["nc"]


def _prep_core(c, hT_bf, cosT_bf, sinT_bf, kbias_np, wq, wk, wv, wo):
    bf = ml_dtypes.bfloat16
    # combined per-k-tile weight blocks: [NKT, 128, 768] = q heads | K | V
    sq = wq[:, c * GROUPS * D:(c + 1) * GROUPS * D].reshape(NKT, 128, GROUPS * D)
    sk = wk[:, c * D:(c + 1) * D].reshape(NKT, 128, D)
    sv = wv[:, c * D:(c + 1) * D].reshape(NKT, 128, D)
    wall = np.ascontiguousarray(
        np.concatenate([sq, sk, sv], axis=2)).astype(bf)
    woh = np.ascontiguousarray(
        wo[c * GROUPS * D:(c + 1) * GROUPS * D, :].reshape(HPC, 128, H)
    ).astype(bf)
    # rope halves-swap is applied to (ps * sin) instead of ps, so the sin
    # table itself must be pre-swapped on the partition axis (involution)
    sinT_sw = np.ascontiguousarray(np.roll(sinT_bf, 64, axis=0))
    sel_np = np.zeros((128, 512), ml_dtypes.bfloat16)
    for k in range(4):
        sel_np[32 * k, k * 128:(k + 1) * 128] = 1.0
    return {
        "hT": hT_bf, "wall": wall, "woh": woh,
        "cosT": cosT_bf, "sinT": sinT_sw, "kbias": kbias_np,
        "seld": sel_np,
    }


def kernel(hidden_states, cos, sin, attention_mask, wq, wk, wv, wo):
    from concourse.bass_utils import run_bass_kernel_spmd

    bf = ml_dtypes.bfloat16
    hidden_states = np.asarray(hidden_states, dtype=np.float32)
    cos = np.asarray(cos, dtype=np.float32)
    sin = np.asarray(sin, dtype=np.float32)
    mask = np.asarray(attention_mask)
    wq = np.asarray(wq, dtype=np.float32)
    wk = np.asarray(wk, dtype=np.float32)
    wv = np.asarray(wv, dtype=np.float32)
    wo = np.asarray(wo, dtype=np.float32)

    h2 = hidden_states.reshape(S2, H)
    hT_bf = np.ascontiguousarray(h2.T).astype(bf)
    cosT_bf = np.ascontiguousarray(
        np.concatenate([cos[b].T for b in range(B)], axis=1)
    ).astype(bf)
    ss = sin.copy()
    ss[..., : D // 2] *= -1.0
    sinT_bf = np.ascontiguousarray(
        np.concatenate([ss[b].T for b in range(B)], axis=1)
    ).astype(bf)
    # padding-mask bias, folded into exp's per-partition bias: [128, b*8+kt]
    kbias_np = np.zeros((128, B * 8), np.float32)
    for b in range(B):
        mb = mask[b].astype(bool)
        for kt in range(8):
            kbias_np[:, b * 8 + kt] = np.where(mb[kt * 128:(kt + 1) * 128], 0.0, NEG)
    kbias_np = np.ascontiguousarray(kbias_np)

    in_maps = [
        _prep_core(c, hT_bf, cosT_bf, sinT_bf, kbias_np, wq, wk, wv, wo)
        for c in range(N_CORES)
    ]
    nc = _get_kernel()
    res = run_bass_kernel_spmd(nc, in_maps, core_ids=list(range(N_CORES)))
    acc = np.zeros((S2, H), np.float64)
    for r in res.results:
        acc += r["out"].astype(np.float64)
    return acc.astype(np.float32).reshape(B, S, H)
